# revision 1
# baseline (speedup 1.0000x reference)
"""Trainium2 Bass kernel for nn_CrossAttention (degenerate cross-attention).

Math (see reference):
    qs_b  = (sum_d x2[b,d] * Wq[d]) / sqrt(128)          # per-batch scalar
    s[b,i]   = x1[b,i] * qs_b
    out[b,i] = sum_j x2[b,j] * exp(s[b,i]*Wk[j]) / sum_j exp(s[b,i]*Wk[j])

Device strategy (pure data parallel, 16 batches per core):
    For each batch b and each 1024-wide chunk of i:
      PE  : outer product (qs_b*Wk)[j] x x1[b,i]      -> PSUM [128, 1024]
      ACT : exp(PSUM)                                  -> SBUF E [128, 1024]
      PE  : [x2_b | ones] reduction over j (partition dim), all 16 batches
            accumulated into one PSUM [32, 1024] (rows lb -> num, 16+lb -> den)
      DVE : shuffle den rows onto partitions 0..15, reciprocal, multiply
"""

import threading

import numpy as np

B = 128
L1 = 8192
DH = 128
NCORES = 8
BPC = B // NCORES  # 16 batches per core
CH = 1024  # free-dim chunk of i per pipeline step
NT = L1 // CH  # 8 chunks

_cache = threading.local()


def _build_module():
    import concourse.bacc as bacc
    import concourse.mybir as mybir
    import concourse.tile as tile

    f32 = mybir.dt.float32
    nc = bacc.Bacc("TRN2", target_bir_lowering=False, debug=False)

    x1p = nc.dram_tensor("x1p", [NT, BPC * CH], f32, kind="ExternalInput").ap()
    wkq = nc.dram_tensor("wkq", [1, BPC * DH], f32, kind="ExternalInput").ap()
    c2p = nc.dram_tensor("c2p", [DH, 2 * BPC * BPC], f32, kind="ExternalInput").ap()
    outp = nc.dram_tensor("outp", [NT, BPC * CH], f32, kind="ExternalOutput").ap()

    EXP = mybir.ActivationFunctionType.Exp
    swap_mask = list(range(16, 32)) + list(range(0, 16))

    with tile.TileContext(nc) as tc:
        with (
            tc.tile_pool(name="const", bufs=1) as const_pool,
            tc.tile_pool(name="stage", bufs=2) as stage_pool,
            tc.tile_pool(name="epool", bufs=2) as epool,
            tc.tile_pool(name="opsum", bufs=2, space="PSUM") as opsum,
            tc.tile_pool(name="rpsum", bufs=2, space="PSUM") as rpsum,
            tc.tile_pool(name="dpool", bufs=2) as dpool,
            tc.tile_pool(name="outpool", bufs=2) as outpool,
        ):
            wkq_sb = const_pool.tile([1, BPC * DH], f32)
            nc.sync.dma_start(wkq_sb[:], wkq[:])
            c2_sb = const_pool.tile([DH, 2 * BPC * BPC], f32)
            nc.sync.dma_start(c2_sb[:], c2p[:])

            for t in range(NT):
                st = stage_pool.tile([1, BPC * CH], f32)
                nc.sync.dma_start(st[:], x1p[t : t + 1, :])

                r_ps = rpsum.tile([2 * BPC, CH], f32)
                for lb in range(BPC):
                    o_ps = opsum.tile([DH, CH], f32)
                    for h in range(CH // 512):
                        nc.tensor.matmul(
                            o_ps[:, h * 512 : (h + 1) * 512],
                            wkq_sb[0:1, lb * DH : (lb + 1) * DH],
                            st[0:1, lb * CH + h * 512 : lb * CH + (h + 1) * 512],
                            start=True,
                            stop=True,
                        )
                    e_sb = epool.tile([DH, CH], f32)
                    nc.scalar.activation(e_sb[:], o_ps[:], EXP)
                    for h in range(CH // 512):
                        nc.tensor.matmul(
                            r_ps[:, h * 512 : (h + 1) * 512],
                            c2_sb[:, lb * 2 * BPC : (lb + 1) * 2 * BPC],
                            e_sb[:, h * 512 : (h + 1) * 512],
                            start=(lb == 0),
                            stop=(lb == BPC - 1),
                        )

                # realign den rows (16..31) onto partitions 0..15, then divide
                rsh = dpool.tile([2 * BPC, CH], f32)
                nc.vector.stream_shuffle(rsh[:], r_ps[:], swap_mask)
                dinv = dpool.tile([BPC, CH], f32)
                nc.vector.reciprocal(dinv[:], rsh[0:BPC, :])
                o_sb = outpool.tile([BPC, CH], f32)
                nc.vector.tensor_mul(o_sb[:], r_ps[0:BPC, :], dinv[:])

                dst = outp[t : t + 1, :].rearrange("a (b n) -> (a b) n", b=BPC)
                nc.sync.dma_start(dst, o_sb[:])

    nc.compile()
    return nc


def _get_module():
    if not hasattr(_cache, "nc"):
        _cache.nc = _build_module()
    return _cache.nc


def kernel(x1, x2, Wq, Wk):
    from concourse.bass_utils import run_bass_kernel_spmd

    x1 = np.asarray(x1, dtype=np.float32)
    x2 = np.asarray(x2, dtype=np.float32)
    Wq = np.asarray(Wq, dtype=np.float32)
    Wk = np.asarray(Wk, dtype=np.float32)

    scale = np.float32(1.0 / np.sqrt(np.float32(DH)))
    qs = (x2 @ Wq) * scale  # [B] f32

    nc = _get_module()

    in_maps = []
    for c in range(NCORES):
        bs = slice(c * BPC, (c + 1) * BPC)
        # x1 permuted: row t holds [x1[b, t*CH:(t+1)*CH] for each local batch lb]
        x1c = x1[bs].reshape(BPC, NT, CH).transpose(1, 0, 2).reshape(NT, BPC * CH)
        # per-batch stationary rows for the outer product: qs_b * Wk
        wkqc = (qs[bs, None] * Wk[None, :]).astype(np.float32).reshape(1, BPC * DH)
        # reduction weights: block lb is [DH, 32]; col lb = x2_b, col 16+lb = 1
        c2c = np.zeros((DH, BPC, 2 * BPC), dtype=np.float32)
        for lb in range(BPC):
            c2c[:, lb, lb] = x2[c * BPC + lb]
            c2c[:, lb, BPC + lb] = 1.0
        in_maps.append(
            {
                "x1p": np.ascontiguousarray(x1c),
                "wkq": wkqc,
                "c2p": c2c.reshape(DH, 2 * BPC * BPC),
            }
        )

    res = run_bass_kernel_spmd(nc, in_maps, list(range(NCORES)))

    out = np.empty((B, L1), dtype=np.float32)
    for c in range(NCORES):
        oc = res.results[c]["outp"]  # [NT, BPC*CH]
        oc = oc.reshape(NT, BPC, CH).transpose(1, 0, 2).reshape(BPC, L1)
        out[c * BPC : (c + 1) * BPC] = oc
    return out
